# revision 9
# baseline (speedup 1.0000x reference)
"""PointNet++ encoder kernel for Trainium2 (8 NeuronCores, data-parallel over batch).

Split of work:
  - Host (numpy, f32-faithful to the jax reference): farthest-point sampling and
    ball-query neighbor selection (sequential / sorting-heavy retrieval steps),
    producing int32 gather-index tables.
  - Device (one SPMD NEFF on 8 cores, 2 batch elements per core): all MLPs
    (feature-major matmuls on the PE), relu+bias on ACT, neighborhood max-pool
    via SWDGE indirect-DMA row gathers + DVE strided reduce_max, final max over
    centroids, i.e. every FLOP-heavy part of the network.

Pointwise-MLP observation: in this architecture mlp() is applied per-point
before each neighborhood max-pool, so the device evaluates each MLP once per
unique point (8192 / 512 / 128 rows) and gathers *features* for pooling,
instead of evaluating on gathered duplicates (25600 / 6400 rows).
"""

import numpy as np

B, N = 16, 8192
NP1, NP2, K = 512, 128, 50
R1, R2 = 0.2, 0.4
NCORES = 8
BPC = B // NCORES  # batches per core


# ----------------------------------------------------------------------------
# Host-side retrieval (mirrors reference.py in float32)
# ----------------------------------------------------------------------------

def _fps(xyz, npoint):
    # xyz [n, d] f32 -> [npoint] int32, deterministic start at 0
    return _fps_batch(xyz[None], npoint)[0]


def _fps_batch(xyz, npoint):
    # xyz [b, n, d] f32 -> [b, npoint] int32 (vectorized over batch; per-batch
    # math identical to the scalar loop)
    b, n, _ = xyz.shape
    distance = np.full((b, n), 1e10, np.float32)
    farthest = np.zeros((b,), np.int64)
    cents = np.empty((b, npoint), np.int32)
    bi = np.arange(b)
    for i in range(npoint):
        cents[:, i] = farthest
        c = xyz[bi, farthest]                       # [b, d]
        d = xyz - c[:, None, :]
        dist = np.sum(d * d, axis=-1, dtype=np.float32)
        distance = np.minimum(distance, dist)
        farthest = np.argmax(distance, axis=1)
    return cents


def _ball_query(cents_xyz, pts, k, radius):
    # cents_xyz [S, d], pts [n, d] -> [S, k] int32 (first k in-radius indices,
    # missing slots padded with n-1, matching the reference's top_k emulation)
    n = pts.shape[0]
    d = cents_xyz[:, None, :] - pts[None, :, :]
    dd = np.sum(d * d, axis=-1, dtype=np.float32)
    mask = dd < np.float32(radius * radius)
    order = np.where(mask, np.arange(n, dtype=np.int32)[None, :], np.int32(n))
    sel = np.sort(order, axis=1)[:, :k]
    return np.where(sel == n, np.int32(n - 1), sel).astype(np.int32)


def _host_mlp(x, params):
    for W, b in params:
        x = np.maximum(x @ W + b, 0.0).astype(np.float32)
    return x


# ----------------------------------------------------------------------------
# Device kernel
# ----------------------------------------------------------------------------

def build_kernel():
    import concourse.bass as bass
    import concourse.mybir as mybir
    import concourse.tile as tile
    from concourse.masks import make_identity

    f32 = mybir.dt.float32
    i32 = mybir.dt.int32
    X = mybir.AxisListType.X
    Relu = mybir.ActivationFunctionType.Relu
    MAX = mybir.AluOpType.max

    nc = bass.Bass()

    ptsT_d = nc.dram_tensor("ptsT", [BPC, 3, N], f32, kind="ExternalInput")
    off1_d = nc.dram_tensor("off1", [BPC, 128, 200], i32, kind="ExternalInput")
    off2_d = nc.dram_tensor("off2", [BPC, 128, K], i32, kind="ExternalInput")
    wdims = [(3, 64), (64, 64), (64, 128),
             (128, 128), (128, 128), (128, 256),
             (256, 256), (256, 512), (512, 1024), (1024, 2048)]
    w_d, b_d = [], []
    for li, (ki, mi) in enumerate(wdims):
        w_d.append(nc.dram_tensor(f"w{li}", [ki, mi], f32, kind="ExternalInput"))
        p = min(mi, 128)
        b_d.append(nc.dram_tensor(f"b{li}", [p, mi // p], f32, kind="ExternalInput"))
    xout_d = nc.dram_tensor("xout", [BPC, 16 * 128], f32, kind="ExternalOutput")

    with tile.TileContext(nc) as tc, \
         tc.tile_pool(name="const", bufs=1) as const, \
         tc.tile_pool(name="work", bufs=2) as work, \
         tc.tile_pool(name="psum", bufs=4, space="PSUM") as psum, \
         tc.tile_pool(name="dram", bufs=1, space="DRAM") as dpool:

        ident = const.tile([128, 128], f32, name="ident")
        make_identity(nc, ident[:])

        # weights/biases resident in SBUF (shared across both batches)
        ws, bs = [], []
        for li, (ki, mi) in enumerate(wdims):
            kt = []
            for kc in range(0, ki, 128):
                kk = min(128, ki - kc)
                t = const.tile([kk, mi], f32, name=f"w{li}_{kc}")
                nc.sync.dma_start(out=t[:], in_=w_d[li][kc:kc + kk, :])
                kt.append(t)
            ws.append(kt)
            p = min(mi, 128)
            t = const.tile([p, mi // p], f32, name=f"b{li}")
            nc.sync.dma_start(out=t[:], in_=b_d[li][:])
            bs.append(t)

        f3_dram = dpool.tile([N, 128], f32, name="f3_dram")
        f2_dram = dpool.tile([NP1, 256], f32, name="f2_dram")

        def mm_act(out_sb, rhs_list, w_tiles, mh, bias_t, n):
            """out_sb[:, :n] = relu( (W[:, mh*128:...]^T @ rhs) + b ), rhs split in K-chunks."""
            ps = psum.tile([128, 512], f32, name="mmps", bufs=4)[:out_sb.shape[0], :n]
            nk = len(rhs_list)
            for kc in range(nk):
                nc.tensor.matmul(
                    out=ps[:],
                    lhsT=w_tiles[kc][:, mh * out_sb.shape[0]:(mh + 1) * out_sb.shape[0]],
                    rhs=rhs_list[kc][:],
                    start=(kc == 0), stop=(kc == nk - 1),
                )
            nc.scalar.activation(out_sb[:], ps[:], Relu,
                                 bias=bias_t[:, mh:mh + 1], scale=1.0)

        for b in range(BPC):
            # ---------------- MLP1 (streamed over 16 x 512-pt chunks) --------
            for t in range(16):
                sl = slice(t * 512, (t + 1) * 512)
                ptsc = work.tile([3, 512], f32, name="ptsc")
                nc.sync.dma_start(out=ptsc[:], in_=ptsT_d[b, :, sl])
                f1c = work.tile([64, 512], f32, name="f1c")
                mm_act(f1c, [ptsc], ws[0], 0, bs[0], 512)
                f2c = work.tile([64, 512], f32, name="f2c")
                mm_act(f2c, [f1c], ws[1], 0, bs[1], 512)
                f3c = work.tile([128, 512], f32, name="f3c")
                mm_act(f3c, [f2c], ws[2], 0, bs[2], 512)
                # transpose 4x [128,128] -> DRAM rows (point-major)
                for u in range(4):
                    tp = psum.tile([128, 128], f32, name="tp", bufs=3)
                    nc.tensor.transpose(out=tp[:], in_=f3c[:, u * 128:(u + 1) * 128],
                                        identity=ident[:])
                    st = work.tile([128, 128], f32, name="st")
                    nc.vector.tensor_copy(out=st[:], in_=tp[:])
                    nc.sync.dma_start(
                        out=f3_dram[t * 512 + u * 128: t * 512 + (u + 1) * 128, :],
                        in_=st[:])

            # ---------------- pool1: gather F3 rows + max over 50 ------------
            # indirect DMA supports exactly one offset per partition per call:
            # call (sh, k) gathers rows idx1[sh*128+p, k] -> [128, 128], then
            # running max over k on DVE.
            off1s = work.tile([128, 200], i32, name="off1s")
            nc.sync.dma_start(out=off1s[:], in_=off1_d[b])
            p3pm = work.tile([128, 4, 128], f32, name="p3pm", bufs=1)  # [s_lo, s_hi, f]
            for sh in range(4):
                for k in range(K):
                    c = k * 4 + sh
                    g1 = work.tile([128, 128], f32, name="g1", bufs=4)
                    nc.gpsimd.indirect_dma_start(
                        out=g1[:], out_offset=None,
                        in_=f3_dram[:],
                        in_offset=bass.IndirectOffsetOnAxis(
                            ap=off1s[:, c:c + 1], axis=0),
                    )
                    if k == 0:
                        nc.vector.tensor_copy(out=p3pm[:, sh, :], in_=g1[:])
                    else:
                        nc.vector.tensor_tensor(out=p3pm[:, sh, :],
                                                in0=p3pm[:, sh, :], in1=g1[:], op=MAX)
            # to feature-major [128f, 512s]
            p3fm = work.tile([128, 512], f32, name="p3fm", bufs=1)
            for sh in range(4):
                tp2 = psum.tile([128, 128], f32, name="tp", bufs=3)
                nc.tensor.transpose(out=tp2[:], in_=p3pm[:, sh, :], identity=ident[:])
                nc.vector.tensor_copy(out=p3fm[:, sh * 128:(sh + 1) * 128], in_=tp2[:])

            # ---------------- MLP2 ------------------------------------------
            f4 = work.tile([128, 512], f32, name="f4", bufs=1)
            mm_act(f4, [p3fm], ws[3], 0, bs[3], 512)
            f5 = work.tile([128, 512], f32, name="f5", bufs=1)
            mm_act(f5, [f4], ws[4], 0, bs[4], 512)
            f6 = [work.tile([128, 512], f32, name=f"f6_{mh}", bufs=1) for mh in range(2)]
            for mh in range(2):
                mm_act(f6[mh], [f5], ws[5], mh, bs[5], 512)
            # store point-major to DRAM [512, 256]
            for sh in range(4):
                for mh in range(2):
                    tp3 = psum.tile([128, 128], f32, name="tp", bufs=3)
                    nc.tensor.transpose(out=tp3[:], in_=f6[mh][:, sh * 128:(sh + 1) * 128],
                                        identity=ident[:])
                    st2 = work.tile([128, 128], f32, name="st2")
                    nc.vector.tensor_copy(out=st2[:], in_=tp3[:])
                    nc.sync.dma_start(
                        out=f2_dram[sh * 128:(sh + 1) * 128, mh * 128:(mh + 1) * 128],
                        in_=st2[:])

            # ---------------- pool2: gather + max over 50 --------------------
            off2s = work.tile([128, K], i32, name="off2s")
            nc.sync.dma_start(out=off2s[:], in_=off2_d[b])
            p4 = work.tile([128, 256], f32, name="p4", bufs=1)  # [s, f] point-major
            for k in range(K):
                g2 = work.tile([128, 256], f32, name="g2", bufs=4)
                nc.gpsimd.indirect_dma_start(
                    out=g2[:], out_offset=None,
                    in_=f2_dram[:],
                    in_offset=bass.IndirectOffsetOnAxis(
                        ap=off2s[:, k:k + 1], axis=0),
                )
                if k == 0:
                    nc.vector.tensor_copy(out=p4[:], in_=g2[:])
                else:
                    nc.vector.tensor_tensor(out=p4[:], in0=p4[:], in1=g2[:], op=MAX)
            p4fm = [work.tile([128, 128], f32, name=f"p4fm_{mh}", bufs=1) for mh in range(2)]
            for mh in range(2):
                tp4 = psum.tile([128, 128], f32, name="tp", bufs=3)
                nc.tensor.transpose(out=tp4[:], in_=p4[:, mh * 128:(mh + 1) * 128],
                                    identity=ident[:])
                nc.vector.tensor_copy(out=p4fm[mh][:], in_=tp4[:])

            # ---------------- MLP3 (N=128 pts on free axis) ------------------
            f7 = [work.tile([128, 128], f32, name=f"f7_{mh}", bufs=1) for mh in range(2)]
            for mh in range(2):
                mm_act(f7[mh], p4fm, ws[6], mh, bs[6], 128)
            f8 = [work.tile([128, 128], f32, name=f"f8_{mh}", bufs=1) for mh in range(4)]
            for mh in range(4):
                mm_act(f8[mh], f7, ws[7], mh, bs[7], 128)
            f9 = [work.tile([128, 128], f32, name=f"f9_{mh}", bufs=1) for mh in range(8)]
            for mh in range(8):
                mm_act(f9[mh], f8, ws[8], mh, bs[8], 128)
            xm = work.tile([128, 16], f32, name="xm", bufs=1)
            for mh in range(16):
                xf = work.tile([128, 128], f32, name="xf", bufs=2)
                mm_act(xf, f9, ws[9], mh, bs[9], 128)
                nc.vector.tensor_reduce(out=xm[:, mh:mh + 1], in_=xf[:], axis=X, op=MAX)
            nc.sync.dma_start(out=xout_d[b].rearrange("(m p) -> p m", p=128), in_=xm[:])

    _legalize_sync(nc, mybir)
    return nc


def _legalize_sync(nc, mybir):
    """The TPB ISA encodes one sem wait + one sem update per instruction; Tile
    emits several. Split extras onto adjacent same-engine NoOps."""
    nid = 0
    for f in nc.m.functions:
        for bb in f.blocks:
            new = []
            for ins in bb.instructions:
                si = ins.sync_info
                pre, post = [], []
                if si is not None:
                    if si.on_wait is not None and len(si.on_wait) > 1:
                        waits = list(si.on_wait)
                        for w in waits[:-1]:
                            nid += 1
                            pre.append(mybir.InstNoOp(
                                name=f"LGW-{nid}", engine=ins.engine, ins=[], outs=[],
                                sync_info=mybir.SyncInfo(on_wait=[w], on_update=[])))
                        si.on_wait = [waits[-1]]
                    if si.on_update is not None and len(si.on_update) > 1:
                        ups = list(si.on_update)
                        for u in ups[1:]:
                            nid += 1
                            post.append(mybir.InstNoOp(
                                name=f"LGU-{nid}", engine=ins.engine, ins=[], outs=[],
                                sync_info=mybir.SyncInfo(on_wait=[], on_update=[u])))
                        si.on_update = [ups[0]]
                new.extend(pre); new.append(ins); new.extend(post)
            bb.instructions = new


# ----------------------------------------------------------------------------
# Entry point
# ----------------------------------------------------------------------------

def kernel(points, young_mod, pois_ratio, force, PoA_v, params1, params2, params3):
    from concourse.bass_utils import run_bass_kernel_spmd

    points = np.asarray(points, np.float32)
    p1 = [(np.asarray(W, np.float32), np.asarray(b, np.float32)) for W, b in params1]
    p2 = [(np.asarray(W, np.float32), np.asarray(b, np.float32)) for W, b in params2]
    p3 = [(np.asarray(W, np.float32), np.asarray(b, np.float32)) for W, b in params3]
    allp = p1 + p2 + p3

    # ---- host retrieval per batch ----
    off1 = np.empty((B, 128, 200), np.int32)
    off2 = np.empty((B, 128, K), np.int32)
    c1all = _fps_batch(points, NP1)                       # [B, 512]
    points3all = np.empty((B, NP1, 128), np.float32)
    for b in range(B):
        pts = points[b]
        idx1 = _ball_query(pts[c1all[b]], pts, K, R1)     # [512, 50]
        F3h = _host_mlp(pts, p1)                          # [8192, 128]
        points3all[b] = F3h[idx1].max(axis=1)             # [512, 128]
        # off1[p, k*4+sh] = idx1[sh*128+p, k]
        off1[b] = idx1.reshape(4, 128, K).transpose(1, 2, 0).reshape(128, 200)
    c2all = _fps_batch(points3all, NP2)                   # [B, 128]
    for b in range(B):
        off2[b] = _ball_query(points3all[b][c2all[b]], points3all[b], K, R2)

    nc = build_kernel()

    in_maps = []
    for c in range(NCORES):
        bsl = slice(c * BPC, (c + 1) * BPC)
        m = {
            "ptsT": np.ascontiguousarray(points[bsl].transpose(0, 2, 1)),
            "off1": np.ascontiguousarray(off1[bsl]),
            "off2": np.ascontiguousarray(off2[bsl]),
        }
        for li, (W, bvec) in enumerate(allp):
            m[f"w{li}"] = np.ascontiguousarray(W)
            mi = W.shape[1]
            p = min(mi, 128)
            m[f"b{li}"] = np.ascontiguousarray(
                bvec.reshape(mi // p, p).T)  # [p, mi//p]
        in_maps.append(m)

    import os as _os
    res = run_bass_kernel_spmd(nc, in_maps, core_ids=list(range(NCORES)),
                               trace=bool(_os.environ.get("BASS_TRACE")))
    if res.exec_time_ns:
        print(f"HW exec time: {res.exec_time_ns} ns")
        if res.instructions_and_trace:
            print("trace:", res.instructions_and_trace[1])
    x = np.concatenate([r["xout"] for r in res.results], axis=0)  # [16, 2048]
    # x[b, mh*128+p] laid out as xm[p, mh] -> already linearized by DMA AP
    mu, sig = x[:, :1024], x[:, 1024:]

    ym = np.asarray(young_mod, np.float32)
    pr = np.asarray(pois_ratio, np.float32)
    fo = np.asarray(force, np.float32)
    pv = np.asarray(PoA_v, np.float32)
    cond_mean = np.concatenate([ym, pr, fo, pv], axis=1)
    ones = np.ones((B, 1), np.float32)
    cond_var = np.concatenate([0.1 * ones, 0.1 * ones, 0.1 * ones, ones], axis=1)
    return (np.concatenate([mu, cond_mean], axis=1),
            np.concatenate([sig, cond_var], axis=1))


# revision 12
# speedup vs baseline: 1.0009x; 1.0009x over previous
"""PointNet++ encoder kernel for Trainium2 (8 NeuronCores, data-parallel over batch).

Split of work:
  - Host (numpy, f32-faithful to the jax reference): farthest-point sampling and
    ball-query neighbor selection (sequential / sorting-heavy retrieval steps),
    producing int32 gather-index tables.
  - Device (one SPMD NEFF on 8 cores, 2 batch elements per core): all MLPs
    (feature-major matmuls on the PE), relu+bias on ACT, neighborhood max-pool
    via SWDGE indirect-DMA row gathers + DVE strided reduce_max, final max over
    centroids, i.e. every FLOP-heavy part of the network.

Pointwise-MLP observation: in this architecture mlp() is applied per-point
before each neighborhood max-pool, so the device evaluates each MLP once per
unique point (8192 / 512 / 128 rows) and gathers *features* for pooling,
instead of evaluating on gathered duplicates (25600 / 6400 rows).
"""

import numpy as np

B, N = 16, 8192
NP1, NP2, K = 512, 128, 50
R1, R2 = 0.2, 0.4
NCORES = 8
BPC = B // NCORES  # batches per core


# ----------------------------------------------------------------------------
# Host-side retrieval (mirrors reference.py in float32)
# ----------------------------------------------------------------------------

def _fps(xyz, npoint):
    # xyz [n, d] f32 -> [npoint] int32, deterministic start at 0
    return _fps_batch(xyz[None], npoint)[0]


def _fps_batch(xyz, npoint):
    # xyz [b, n, d] f32 -> [b, npoint] int32 (vectorized over batch; per-batch
    # math identical to the scalar loop)
    b, n, _ = xyz.shape
    distance = np.full((b, n), 1e10, np.float32)
    farthest = np.zeros((b,), np.int64)
    cents = np.empty((b, npoint), np.int32)
    bi = np.arange(b)
    for i in range(npoint):
        cents[:, i] = farthest
        c = xyz[bi, farthest]                       # [b, d]
        d = xyz - c[:, None, :]
        dist = np.sum(d * d, axis=-1, dtype=np.float32)
        distance = np.minimum(distance, dist)
        farthest = np.argmax(distance, axis=1)
    return cents


def _ball_query(cents_xyz, pts, k, radius):
    # cents_xyz [S, d], pts [n, d] -> [S, k] int32 (first k in-radius indices,
    # missing slots padded with n-1, matching the reference's top_k emulation)
    n = pts.shape[0]
    d = cents_xyz[:, None, :] - pts[None, :, :]
    dd = np.sum(d * d, axis=-1, dtype=np.float32)
    mask = dd < np.float32(radius * radius)
    order = np.where(mask, np.arange(n, dtype=np.int32)[None, :], np.int32(n))
    sel = np.sort(order, axis=1)[:, :k]
    return np.where(sel == n, np.int32(n - 1), sel).astype(np.int32)


def _host_mlp(x, params):
    for W, b in params:
        x = np.maximum(x @ W + b, 0.0).astype(np.float32)
    return x


# ----------------------------------------------------------------------------
# Device kernel
# ----------------------------------------------------------------------------

def build_kernel():
    import concourse.bass as bass
    import concourse.mybir as mybir
    import concourse.tile as tile
    from concourse.masks import make_identity

    f32 = mybir.dt.float32
    i32 = mybir.dt.int32
    X = mybir.AxisListType.X
    Relu = mybir.ActivationFunctionType.Relu
    MAX = mybir.AluOpType.max

    nc = bass.Bass()

    ptsT_d = nc.dram_tensor("ptsT", [BPC, 3, N], f32, kind="ExternalInput")
    off1_d = nc.dram_tensor("off1", [BPC, 128, 200], i32, kind="ExternalInput")
    off2_d = nc.dram_tensor("off2", [BPC, 128, K], i32, kind="ExternalInput")
    wdims = [(3, 64), (64, 64), (64, 128),
             (128, 128), (128, 128), (128, 256),
             (256, 256), (256, 512), (512, 1024), (1024, 2048)]
    w_d, b_d = [], []
    for li, (ki, mi) in enumerate(wdims):
        w_d.append(nc.dram_tensor(f"w{li}", [ki, mi], f32, kind="ExternalInput"))
        p = min(mi, 128)
        b_d.append(nc.dram_tensor(f"b{li}", [p, mi // p], f32, kind="ExternalInput"))
    xout_d = nc.dram_tensor("xout", [BPC, 16 * 128], f32, kind="ExternalOutput")

    with tile.TileContext(nc) as tc, \
         tc.tile_pool(name="const", bufs=1) as const, \
         tc.tile_pool(name="work", bufs=2) as work, \
         tc.tile_pool(name="psum", bufs=4, space="PSUM") as psum, \
         tc.tile_pool(name="dram", bufs=1, space="DRAM") as dpool:

        ident = const.tile([128, 128], f32, name="ident")
        make_identity(nc, ident[:])

        # weights/biases resident in SBUF (shared across both batches)
        ws, bs = [], []
        for li, (ki, mi) in enumerate(wdims):
            kt = []
            for kc in range(0, ki, 128):
                kk = min(128, ki - kc)
                t = const.tile([kk, mi], f32, name=f"w{li}_{kc}")
                nc.sync.dma_start(out=t[:], in_=w_d[li][kc:kc + kk, :])
                kt.append(t)
            ws.append(kt)
            p = min(mi, 128)
            t = const.tile([p, mi // p], f32, name=f"b{li}")
            nc.sync.dma_start(out=t[:], in_=b_d[li][:])
            bs.append(t)

        f3_drams = [dpool.tile([N, 128], f32, name=f"f3_dram_{b}")
                    for b in range(BPC)]
        f2_drams = [dpool.tile([NP1, 256], f32, name=f"f2_dram_{b}")
                    for b in range(BPC)]

        def mm_act(out_sb, rhs_list, w_tiles, mh, bias_t, n):
            """out_sb[:, :n] = relu( (W[:, mh*128:...]^T @ rhs) + b ), rhs split in K-chunks."""
            ps = psum.tile([128, 512], f32, name="mmps", bufs=4)[:out_sb.shape[0], :n]
            nk = len(rhs_list)
            for kc in range(nk):
                nc.tensor.matmul(
                    out=ps[:],
                    lhsT=w_tiles[kc][:, mh * out_sb.shape[0]:(mh + 1) * out_sb.shape[0]],
                    rhs=rhs_list[kc][:],
                    start=(kc == 0), stop=(kc == nk - 1),
                )
            nc.scalar.activation(out_sb[:], ps[:], Relu,
                                 bias=bias_t[:, mh:mh + 1], scale=1.0)

        for b in range(BPC):
            f3_dram = f3_drams[b]
            f2_dram = f2_drams[b]
            # ---------------- MLP1 (streamed over 16 x 512-pt chunks) --------
            for t in range(16):
                sl = slice(t * 512, (t + 1) * 512)
                ptsc = work.tile([3, 512], f32, name="ptsc")
                nc.sync.dma_start(out=ptsc[:], in_=ptsT_d[b, :, sl])
                f1c = work.tile([64, 512], f32, name="f1c")
                mm_act(f1c, [ptsc], ws[0], 0, bs[0], 512)
                f2c = work.tile([64, 512], f32, name="f2c")
                mm_act(f2c, [f1c], ws[1], 0, bs[1], 512)
                f3c = work.tile([128, 512], f32, name="f3c")
                mm_act(f3c, [f2c], ws[2], 0, bs[2], 512)
                # transpose 4x [128,128] -> DRAM rows (point-major)
                for u in range(4):
                    tp = psum.tile([128, 128], f32, name="tp", bufs=3)
                    nc.tensor.transpose(out=tp[:], in_=f3c[:, u * 128:(u + 1) * 128],
                                        identity=ident[:])
                    st = work.tile([128, 128], f32, name="st")
                    nc.vector.tensor_copy(out=st[:], in_=tp[:])
                    nc.sync.dma_start(
                        out=f3_dram[t * 512 + u * 128: t * 512 + (u + 1) * 128, :],
                        in_=st[:])

            # ---------------- pool1: gather F3 rows + max over 50 ------------
            # indirect DMA supports exactly one offset per partition per call:
            # call (sh, k) gathers rows idx1[sh*128+p, k] -> [128, 128], then
            # running max over k on DVE.
            off1s = work.tile([128, 200], i32, name="off1s")
            nc.sync.dma_start(out=off1s[:], in_=off1_d[b])
            p3pm = work.tile([128, 4, 128], f32, name="p3pm", bufs=1)  # [s_lo, s_hi, f]
            for sh in range(4):
                for k in range(K):
                    c = k * 4 + sh
                    g1 = work.tile([128, 128], f32, name="g1", bufs=8)
                    nc.gpsimd.indirect_dma_start(
                        out=g1[:], out_offset=None,
                        in_=f3_dram[:],
                        in_offset=bass.IndirectOffsetOnAxis(
                            ap=off1s[:, c:c + 1], axis=0),
                    )
                    if k == 0:
                        nc.vector.tensor_copy(out=p3pm[:, sh, :], in_=g1[:])
                    else:
                        nc.vector.tensor_tensor(out=p3pm[:, sh, :],
                                                in0=p3pm[:, sh, :], in1=g1[:], op=MAX)
            # to feature-major [128f, 512s]
            p3fm = work.tile([128, 512], f32, name="p3fm", bufs=1)
            for sh in range(4):
                tp2 = psum.tile([128, 128], f32, name="tp", bufs=3)
                nc.tensor.transpose(out=tp2[:], in_=p3pm[:, sh, :], identity=ident[:])
                nc.vector.tensor_copy(out=p3fm[:, sh * 128:(sh + 1) * 128], in_=tp2[:])

            # ---------------- MLP2 ------------------------------------------
            f4 = work.tile([128, 512], f32, name="f4", bufs=1)
            mm_act(f4, [p3fm], ws[3], 0, bs[3], 512)
            f5 = work.tile([128, 512], f32, name="f5", bufs=1)
            mm_act(f5, [f4], ws[4], 0, bs[4], 512)
            f6 = [work.tile([128, 512], f32, name=f"f6_{mh}", bufs=1) for mh in range(2)]
            for mh in range(2):
                mm_act(f6[mh], [f5], ws[5], mh, bs[5], 512)
            # store point-major to DRAM [512, 256]
            for sh in range(4):
                for mh in range(2):
                    tp3 = psum.tile([128, 128], f32, name="tp", bufs=3)
                    nc.tensor.transpose(out=tp3[:], in_=f6[mh][:, sh * 128:(sh + 1) * 128],
                                        identity=ident[:])
                    st2 = work.tile([128, 128], f32, name="st2")
                    nc.vector.tensor_copy(out=st2[:], in_=tp3[:])
                    nc.sync.dma_start(
                        out=f2_dram[sh * 128:(sh + 1) * 128, mh * 128:(mh + 1) * 128],
                        in_=st2[:])

            # ---------------- pool2: gather + max over 50 --------------------
            off2s = work.tile([128, K], i32, name="off2s")
            nc.sync.dma_start(out=off2s[:], in_=off2_d[b])
            p4 = work.tile([128, 256], f32, name="p4", bufs=1)  # [s, f] point-major
            for k in range(K):
                g2 = work.tile([128, 256], f32, name="g2", bufs=8)
                nc.gpsimd.indirect_dma_start(
                    out=g2[:], out_offset=None,
                    in_=f2_dram[:],
                    in_offset=bass.IndirectOffsetOnAxis(
                        ap=off2s[:, k:k + 1], axis=0),
                )
                if k == 0:
                    nc.vector.tensor_copy(out=p4[:], in_=g2[:])
                else:
                    nc.vector.tensor_tensor(out=p4[:], in0=p4[:], in1=g2[:], op=MAX)
            p4fm = [work.tile([128, 128], f32, name=f"p4fm_{mh}", bufs=1) for mh in range(2)]
            for mh in range(2):
                tp4 = psum.tile([128, 128], f32, name="tp", bufs=3)
                nc.tensor.transpose(out=tp4[:], in_=p4[:, mh * 128:(mh + 1) * 128],
                                    identity=ident[:])
                nc.vector.tensor_copy(out=p4fm[mh][:], in_=tp4[:])

            # ---------------- MLP3 (N=128 pts on free axis) ------------------
            f7 = [work.tile([128, 128], f32, name=f"f7_{mh}", bufs=1) for mh in range(2)]
            for mh in range(2):
                mm_act(f7[mh], p4fm, ws[6], mh, bs[6], 128)
            f8 = [work.tile([128, 128], f32, name=f"f8_{mh}", bufs=1) for mh in range(4)]
            for mh in range(4):
                mm_act(f8[mh], f7, ws[7], mh, bs[7], 128)
            f9 = [work.tile([128, 128], f32, name=f"f9_{mh}", bufs=1) for mh in range(8)]
            for mh in range(8):
                mm_act(f9[mh], f8, ws[8], mh, bs[8], 128)
            xm = work.tile([128, 16], f32, name="xm", bufs=1)
            for mh in range(16):
                xf = work.tile([128, 128], f32, name="xf", bufs=2)
                mm_act(xf, f9, ws[9], mh, bs[9], 128)
                nc.vector.tensor_reduce(out=xm[:, mh:mh + 1], in_=xf[:], axis=X, op=MAX)
            nc.sync.dma_start(out=xout_d[b].rearrange("(m p) -> p m", p=128), in_=xm[:])

    _legalize_sync(nc, mybir)
    return nc


def _legalize_sync(nc, mybir):
    """The TPB ISA encodes one sem wait + one sem update per instruction; Tile
    emits several. Split extras onto adjacent same-engine NoOps."""
    nid = 0
    for f in nc.m.functions:
        for bb in f.blocks:
            new = []
            for ins in bb.instructions:
                si = ins.sync_info
                pre, post = [], []
                if si is not None:
                    if si.on_wait is not None and len(si.on_wait) > 1:
                        waits = list(si.on_wait)
                        for w in waits[:-1]:
                            nid += 1
                            pre.append(mybir.InstNoOp(
                                name=f"LGW-{nid}", engine=ins.engine, ins=[], outs=[],
                                sync_info=mybir.SyncInfo(on_wait=[w], on_update=[])))
                        si.on_wait = [waits[-1]]
                    if si.on_update is not None and len(si.on_update) > 1:
                        ups = list(si.on_update)
                        for u in ups[1:]:
                            nid += 1
                            post.append(mybir.InstNoOp(
                                name=f"LGU-{nid}", engine=ins.engine, ins=[], outs=[],
                                sync_info=mybir.SyncInfo(on_wait=[], on_update=[u])))
                        si.on_update = [ups[0]]
                new.extend(pre); new.append(ins); new.extend(post)
            bb.instructions = new


# ----------------------------------------------------------------------------
# Entry point
# ----------------------------------------------------------------------------

def kernel(points, young_mod, pois_ratio, force, PoA_v, params1, params2, params3):
    from concourse.bass_utils import run_bass_kernel_spmd

    points = np.asarray(points, np.float32)
    p1 = [(np.asarray(W, np.float32), np.asarray(b, np.float32)) for W, b in params1]
    p2 = [(np.asarray(W, np.float32), np.asarray(b, np.float32)) for W, b in params2]
    p3 = [(np.asarray(W, np.float32), np.asarray(b, np.float32)) for W, b in params3]
    allp = p1 + p2 + p3

    # ---- host retrieval per batch ----
    off1 = np.empty((B, 128, 200), np.int32)
    off2 = np.empty((B, 128, K), np.int32)
    c1all = _fps_batch(points, NP1)                       # [B, 512]
    points3all = np.empty((B, NP1, 128), np.float32)
    for b in range(B):
        pts = points[b]
        idx1 = _ball_query(pts[c1all[b]], pts, K, R1)     # [512, 50]
        F3h = _host_mlp(pts, p1)                          # [8192, 128]
        points3all[b] = F3h[idx1].max(axis=1)             # [512, 128]
        # off1[p, k*4+sh] = idx1[sh*128+p, k]
        off1[b] = idx1.reshape(4, 128, K).transpose(1, 2, 0).reshape(128, 200)
    c2all = _fps_batch(points3all, NP2)                   # [B, 128]
    for b in range(B):
        off2[b] = _ball_query(points3all[b][c2all[b]], points3all[b], K, R2)

    nc = build_kernel()

    in_maps = []
    for c in range(NCORES):
        bsl = slice(c * BPC, (c + 1) * BPC)
        m = {
            "ptsT": np.ascontiguousarray(points[bsl].transpose(0, 2, 1)),
            "off1": np.ascontiguousarray(off1[bsl]),
            "off2": np.ascontiguousarray(off2[bsl]),
        }
        for li, (W, bvec) in enumerate(allp):
            m[f"w{li}"] = np.ascontiguousarray(W)
            mi = W.shape[1]
            p = min(mi, 128)
            m[f"b{li}"] = np.ascontiguousarray(
                bvec.reshape(mi // p, p).T)  # [p, mi//p]
        in_maps.append(m)

    import os as _os
    try:
        res = run_bass_kernel_spmd(nc, in_maps, core_ids=list(range(NCORES)),
                                   trace=bool(_os.environ.get("BASS_TRACE")))
    except ModuleNotFoundError:
        # NTFF profiling hook unavailable in this container; run untraced.
        _os.environ["BASS_NEVER_TRACE"] = "1"
        res = run_bass_kernel_spmd(nc, in_maps, core_ids=list(range(NCORES)))
    if res.exec_time_ns:
        print(f"HW exec time: {res.exec_time_ns} ns")
        if res.instructions_and_trace:
            print("trace:", res.instructions_and_trace[1])
    x = np.concatenate([r["xout"] for r in res.results], axis=0)  # [16, 2048]
    # x[b, mh*128+p] laid out as xm[p, mh] -> already linearized by DMA AP
    mu, sig = x[:, :1024], x[:, 1024:]

    ym = np.asarray(young_mod, np.float32)
    pr = np.asarray(pois_ratio, np.float32)
    fo = np.asarray(force, np.float32)
    pv = np.asarray(PoA_v, np.float32)
    cond_mean = np.concatenate([ym, pr, fo, pv], axis=1)
    ones = np.ones((B, 1), np.float32)
    cond_var = np.concatenate([0.1 * ones, 0.1 * ones, 0.1 * ones, ones], axis=1)
    return (np.concatenate([mu, cond_mean], axis=1),
            np.concatenate([sig, cond_var], axis=1))
